# revision 28
# baseline (speedup 1.0000x reference)
"""Sharded kNN (retrieval) kernel for 8 Trainium2 NeuronCores — v2.

Strategy (classic sharded-kNN reduction, heavily restructured vs v1):
  - Shard X_train / Y_train along N across 8 cores (12500 rows each).
  - Each core computes scores s[b, n] = x_b . t_n - |t_n|^2/2 for its shard
    (argmax of s  <=>  argmin of euclidean distance) via fp16 matmuls on the
    tensor engine.
  - v2 loop order: candidate-block outer (7 blocks of 2000, padded to 2048),
    query-tile inner — X_train streams from HBM exactly ONCE (v1 streamed it
    8x).  Scores are never materialized in SBUF: the DVE MAX8 / FIND_INDEX8
    instructions scan the 4 PSUM banks of each (block, query-tile) directly.
  - Per-block top-8 candidates are merged across blocks by packing the
    bank-padded candidate id (14 bits, < 7*2048 = 14336) into the low
    mantissa bits of the fp32 score (perturbation ~0.2 << top-1/top-2 score
    gap ~7), which makes every merged value unique and lets the final top-8
    positions be recovered with a single bitwise AND - no per-partition
    gather needed.
  - The final 8 candidates per query are gathered from an id-padded DRAM
    table with one batched indirect DMA and re-ranked exactly in fp32
    (identical tensor_tensor dot products + tie-break as v1, so the final
    ordering matches the reference bit-for-bit on ties).
  - Each core outputs (exact best score, padded local argmin) per query; the
    host does the tiny 8-way (min, argmin) reduction and gathers Y_train.
"""

import numpy as np
from contextlib import ExitStack

import concourse.bass as bass
import concourse.mybir as mybir
import concourse.tile as tile
from concourse.bass_utils import run_bass_kernel_spmd

# Problem shape (hardcoded per contest contract).
N_CORES = 8
B = 1024          # queries
D = 768           # feature dim (48*16)
N = 100000        # training rows
N_LOC = N // N_CORES          # 12500 rows per core
P = 128                       # partitions
BT = B // P                   # 8 query tiles
KC = 128                      # contraction tile
KCHUNKS = D // KC             # 6
NBANK = 500                   # real candidates per PSUM bank
BANKPAD = 512                 # PSUM bank stride in fp32 elements (2 KiB)
BLK = 4 * NBANK               # candidates per full block (4 banks)
NBLK = 7                      # 6 full blocks of 2000 + 1 tail block of 500
BLKPAD = 4 * BANKPAD          # padded ids per block (2048)
NPAD = NBLK * BLKPAD          # padded id space (14336 < 2^14)
TOPK = 8
PAD = 776                     # 768 + 1 (t2/2) + 7 zero pad -> 3104B rows
NEG = -60000.0                # pad-score sentinel (fp16-representable)
# Tie-break sentinel: must stay exactly representable in fp32 when combined
# with padded ids < NPAD (so idx - BIG is exact), i.e. well under 2^24.
BIG = 1.0e6
IDMASK = 0x3FFF               # low 14 bits: padded candidate id
VALMASK = 0xFFFFC000          # high bits: fp32 sign/exponent/upper mantissa

_F16 = mybir.dt.float16
_F32 = mybir.dt.float32
_U32 = mybir.dt.uint32

# candidates per bank b (tail block has a single 500-wide bank)
_BANKS = [4] * 6 + [1]


def _split_sync_waits(nc, maxw=1):
    """Workaround for this walrus build: it accepts at most ONE sync-wait
    command per instruction.  Move extra sem waits onto preceding same-engine
    nops (same queue => executed in order before the instruction)."""
    from bass_rust import InstNoOp

    n_split = 0
    for f in nc.m.functions:
        for blk in f.blocks:
            insts = blk.instructions
            i = 0
            while i < len(insts):
                inst = insts[i]
                si = inst.sync_info
                ow = list(si.on_wait) if (si is not None and si.on_wait) else []
                if len(ow) > maxw:
                    keep, extra = ow[-maxw:], ow[:-maxw]
                    inst.sync_info = mybir.SyncInfo(
                        on_wait=keep, on_update=list(si.on_update or [])
                    )
                    nops = []
                    for j in range(0, len(extra), maxw):
                        nop = InstNoOp(name=f"{inst.name}-ws{j}", ins=[], outs=[])
                        nop.engine = inst.engine
                        nop.sync_info = mybir.SyncInfo(
                            on_wait=extra[j : j + maxw], on_update=[]
                        )
                        nops.append(nop)
                    insts[i:i] = nops
                    i += len(nops)
                    n_split += 1
                i += 1
    return n_split


def _build(iters=1, ablate=""):
    """iters>1 repeats the whole pipeline (identical work) inside one NEFF —
    used by the harness to measure true on-HW time differentially.
    ablate: "" full kernel; "noscan" replaces the DVE max8/find_index8 with
    memsets; "mm" additionally drops merge/gather/re-rank (outputs dummy);
    both are for bottleneck attribution only (results are wrong)."""
    nc = bass.Bass()
    xq16 = nc.dram_tensor("xq16", [P, KCHUNKS * B], _F16, kind="ExternalInput")
    xe32 = nc.dram_tensor("xe32", [P, BT * PAD], _F32, kind="ExternalInput")
    xtr16 = nc.dram_tensor("xtr16", [NBLK, P, KCHUNKS * BLKPAD], _F16,
                           kind="ExternalInput")
    xg32 = nc.dram_tensor("xg32", [NPAD, PAD], _F32, kind="ExternalInput")
    out_val = nc.dram_tensor("out_val", [B, 1], _F32, kind="ExternalOutput")
    out_idx = nc.dram_tensor("out_idx", [B, 1], _F32, kind="ExternalOutput")

    with ExitStack() as ctx:
        tc = ctx.enter_context(tile.TileContext(nc))
        const_pool = ctx.enter_context(tc.tile_pool(name="const", bufs=1))
        xtr_pool = ctx.enter_context(tc.tile_pool(name="xtr", bufs=2))
        s16_pool = ctx.enter_context(tc.tile_pool(name="s16", bufs=3))
        top_pool = ctx.enter_context(tc.tile_pool(name="top", bufs=2))
        fin_pool = ctx.enter_context(tc.tile_pool(name="fin", bufs=2))
        gather_pool = ctx.enter_context(tc.tile_pool(name="gather", bufs=2))
        psum_pool = ctx.enter_context(tc.tile_pool(name="psum", bufs=2, space="PSUM"))

        # queries, stationary: [p, k, b] = x_flat[b, k*128+p]
        xq = const_pool.tile([P, KCHUNKS, B], _F16)
        nc.sync.dma_start(xq[:], xq16[:, :])
        # exact fp32 queries (plus -1 marker) for the re-rank
        xe = const_pool.tile([P, BT, PAD], _F32)
        nc.sync.dma_start(xe[:], xe32[:, :])
        # packed (score | id) candidates: [p, bt, blk, 8]
        cand = const_pool.tile([P, BT, NBLK, TOPK], _F32)

        for _rep in range(iters):
            _body(nc, tc, locals(), ablate)

    _split_waits_maybe(nc)
    return nc


def _body(nc, tc, env, ablate=""):
    xq = env["xq"]; xe = env["xe"]
    cand = env["cand"]; xtr16 = env["xtr16"]; xg32 = env["xg32"]
    out_val = env["out_val"]; out_idx = env["out_idx"]
    xtr_pool = env["xtr_pool"]; top_pool = env["top_pool"]
    fin_pool = env["fin_pool"]; gather_pool = env["gather_pool"]
    psum_pool = env["psum_pool"]; s16_pool = env["s16_pool"]
    if True:
        for b in range(NBLK):
            nbank = _BANKS[b]
            xtr = xtr_pool.tile([P, KCHUNKS, BLKPAD], _F16)
            nc.sync.dma_start(xtr[:], xtr16[b, :, :])
            for bt in range(BT):
                bs = slice(bt * P, (bt + 1) * P)
                ps = psum_pool.tile([P, 4, BANKPAD], _F32)
                # k outer: 4 consecutive matmuls share the same stationary
                # weights (one LDWEIGHTS per k-chunk instead of per matmul).
                # The t^2/2 bias rides in xtr's k=5 row 127 (query side 1.0),
                # which also writes the NEG sentinel into the 12 pad columns.
                for k in range(KCHUNKS):
                    for c in range(nbank):
                        nc.tensor.matmul(
                            ps[:, c, :],
                            lhsT=xq[:, k, bs],
                            rhs=xtr[:, k, c * BANKPAD : (c + 1) * BANKPAD],
                            start=(k == 0),
                            stop=(k == KCHUNKS - 1),
                        )
                # ACT+DVE split-drain PSUM to fp16 SBUF (halves each engine's
                # PSUM-read exposure vs the PE, which contends on the port),
                # then DVE scans SBUF only
                s16 = s16_pool.tile([P, 4, BANKPAD], _F16)
                h = max(nbank // 2, 1)
                nc.scalar.copy(s16[:, 0:h, :], ps[:, 0:h, :])
                if nbank > h:
                    nc.vector.tensor_copy(s16[:, h:nbank, :], ps[:, h:nbank, :])
                scan = s16[:, 0:nbank, :].rearrange("p a b -> p (a b)")
                tv = top_pool.tile([P, TOPK], _F16)
                ti = top_pool.tile([P, TOPK], _U32)
                if ablate:
                    nc.vector.memset(tv[:], 1.0)
                    nc.vector.memset(ti[:], 3)
                else:
                    nc.vector.max(tv[:], scan)
                    nc.vector.max_index(ti[:], tv[:], scan)
                # fp16 -> fp32 (exact); low 13 mantissa bits land as zero
                tvf = top_pool.tile([P, TOPK], _F32)
                nc.vector.tensor_copy(tvf[:], tv[:])
                # global padded id = in-block id | (b * 2048)
                tg = top_pool.tile([P, TOPK], _U32)
                nc.vector.tensor_scalar(
                    tg[:], ti[:], b * BLKPAD, None,
                    op0=mybir.AluOpType.bitwise_or,
                )
                # clear the low 14 mantissa bits of the fp32 score (>>14<<14)
                vm = top_pool.tile([P, TOPK], _U32)
                nc.vector.tensor_scalar(
                    vm[:], tvf[:].bitcast(_U32), 14, 14,
                    op0=mybir.AluOpType.logical_shift_right,
                    op1=mybir.AluOpType.logical_shift_left,
                )
                # pack the id into those bits
                nc.vector.tensor_tensor(
                    cand[:, bt, b, :].bitcast(_U32), vm[:], tg[:],
                    op=mybir.AluOpType.bitwise_or,
                )

        for bt in range(BT):
            bs = slice(bt * P, (bt + 1) * P)
            if ablate == "mm":
                bv = fin_pool.tile([P, 1], _F32)
                nc.vector.memset(bv[:], 0.0)
                bi = fin_pool.tile([P, 1], _F32)
                nc.vector.memset(bi[:], 0.0)
                nc.sync.dma_start(out_val[bs, :], bv[:])
                nc.sync.dma_start(out_idx[bs, :], bi[:])
                continue
            # merge: top-8 of the 56 packed (score|id) values; ids make them
            # unique so FIND_INDEX8 duplicate semantics never matter here
            tp = fin_pool.tile([P, TOPK], _F32)
            nc.vector.max(tp[:], cand[:, bt, :, :].rearrange("p a b -> p (a b)"))
            idx8 = fin_pool.tile([P, TOPK], _U32)
            nc.vector.tensor_scalar(
                idx8[:], tp[:].bitcast(_U32), 18, 18,
                op0=mybir.AluOpType.logical_shift_left,
                op1=mybir.AluOpType.logical_shift_right,
            )

            # gather the 8 candidate rows (768 feats + t2/2 + pad) per query
            # (one indirect DMA per slot: HW SWDGE mis-gathers [128,8] offsets)
            xg = gather_pool.tile([P, TOPK, PAD], _F32)
            for j in range(TOPK):
                nc.gpsimd.indirect_dma_start(
                    out=xg[:, j, :],
                    out_offset=None,
                    in_=xg32[:, :],
                    in_offset=bass.IndirectOffsetOnAxis(ap=idx8[:, j : j + 1], axis=0),
                )

            # exact fp32 re-rank: cand8[j] = xe . xg[j] = x.t - t2/2
            cand8 = fin_pool.tile([P, TOPK], _F32)
            scratch = gather_pool.tile([P, PAD], _F32)
            for j in range(TOPK):
                nc.vector.scalar_tensor_tensor(
                    out=scratch[:],
                    in0=xg[:, j, :],
                    scalar=0.0,
                    in1=xe[:, bt, :],
                    op0=mybir.AluOpType.add,
                    op1=mybir.AluOpType.mult,
                    accum_out=cand8[:, j : j + 1],
                )

            bv = fin_pool.tile([P, 1], _F32)
            nc.vector.tensor_reduce(
                bv[:], cand8[:], axis=mybir.AxisListType.X, op=mybir.AluOpType.max
            )
            # pick the smallest padded id among exact-score ties
            tif = fin_pool.tile([P, TOPK], _F32)
            nc.vector.tensor_copy(tif[:], idx8[:])
            eq = fin_pool.tile([P, TOPK], _F32)
            nc.vector.tensor_scalar(
                eq[:], cand8[:], bv[:], None, op0=mybir.AluOpType.is_equal
            )
            t1 = fin_pool.tile([P, TOPK], _F32)
            nc.vector.scalar_tensor_tensor(
                t1[:],
                in0=tif[:],
                scalar=BIG,
                in1=eq[:],
                op0=mybir.AluOpType.subtract,
                op1=mybir.AluOpType.mult,
            )
            masked = fin_pool.tile([P, TOPK], _F32)
            nc.vector.tensor_scalar_add(masked[:], t1[:], BIG)
            bi = fin_pool.tile([P, 1], _F32)
            nc.vector.tensor_reduce(
                bi[:], masked[:], axis=mybir.AxisListType.X, op=mybir.AluOpType.min
            )

            nc.sync.dma_start(out_val[bs, :], bv[:])
            nc.sync.dma_start(out_idx[bs, :], bi[:])


def _split_waits_maybe(nc):
    import os
    if not os.environ.get("BASS_NO_SPLIT_WAITS"):
        _split_sync_waits(nc)


_NC_CACHE = None
LAST_RESULTS = None  # BassKernelResults of the most recent run (for test harness)

# map padded id -> local row: n = 2000*(g//2048) + 500*((g%2048)//512) + g%512
def _unpad_ids(g):
    g = np.asarray(g, dtype=np.int64)
    blk, rem = np.divmod(g, BLKPAD)
    c, i = np.divmod(rem, BANKPAD)
    return blk * BLK + c * NBANK + i


def prepare_in_maps(x, X_train):
    x = np.asarray(x, dtype=np.float32)
    X_train = np.asarray(X_train, dtype=np.float32)

    x_flat = np.ascontiguousarray(x.reshape(B, D))
    xt16 = x_flat.astype(np.float16)  # [B, D]
    # [p, k, b] = x16[b, k*128+p]; slot (k=5, p=127) carries the bias
    # constant 1.0 instead of feature 767 (dropped from the selection score;
    # the exact re-rank still uses all 768 features)
    xq16 = np.ascontiguousarray(
        xt16.reshape(B, KCHUNKS, P).transpose(2, 1, 0)
    )
    xq16[P - 1, KCHUNKS - 1, :] = np.float16(1.0)
    xq16 = xq16.reshape(P, KCHUNKS * B)
    # [p, bt, d] = xe[bt*128+p, d]
    xe = np.concatenate(
        [x_flat, -np.ones((B, 1), np.float32), np.zeros((B, PAD - D - 1), np.float32)],
        axis=1,
    )
    xe32 = np.ascontiguousarray(
        xe.reshape(BT, P, PAD).transpose(1, 0, 2)
    ).reshape(P, BT * PAD)

    # local row n -> padded id
    n_loc = np.arange(N_LOC)
    blk, rem = np.divmod(n_loc, BLK)
    c, i = np.divmod(rem, NBANK)
    gids = blk * BLKPAD + c * BANKPAD + i  # [N_LOC]

    in_maps = []
    for core in range(N_CORES):
        Xc = X_train[core * N_LOC : (core + 1) * N_LOC]
        t2 = (Xc.astype(np.float64) ** 2).sum(axis=1)
        X16 = Xc.astype(np.float16)  # [N_LOC, D]

        # xtr16[b, p, k*2048 + g] = X16[n(b,g), k*128+p], pad slots zero.
        # Slot (k=5, p=127) carries the bias row (t2.mean - t2)/2 in place of
        # feature 767, with the NEG sentinel in the pad columns.
        xtr = np.zeros((NBLK, P, KCHUNKS, BLKPAD), np.float16)
        x16v = X16.reshape(N_LOC, KCHUNKS, P)  # [n, k, p]
        xtr[blk, :, :, rem // NBANK * BANKPAD + rem % NBANK] = x16v.transpose(0, 2, 1)
        vrow = np.full((NBLK, BLKPAD), NEG, np.float16).reshape(-1)
        vrow[gids] = ((t2.mean() - t2) * 0.5).astype(np.float16)
        xtr[:, P - 1, KCHUNKS - 1, :] = vrow.reshape(NBLK, BLKPAD)
        xtr16 = np.ascontiguousarray(xtr).reshape(NBLK, P, KCHUNKS * BLKPAD)

        xg32 = np.zeros((NPAD, PAD), np.float32)
        xg32[gids, :D] = Xc
        xg32[gids, D] = (t2 * 0.5).astype(np.float32)

        in_maps.append(
            {
                "xq16": xq16,
                "xe32": xe32,
                "xtr16": xtr16,
                "xg32": np.ascontiguousarray(xg32),
            }
        )
    return in_maps


def kernel(x, X_train, Y_train):
    global _NC_CACHE, LAST_RESULTS
    Y_train = np.asarray(Y_train)
    in_maps = prepare_in_maps(x, X_train)

    if _NC_CACHE is None:
        _NC_CACHE = _build()

    LAST_RESULTS = run_bass_kernel_spmd(
        _NC_CACHE,
        in_maps,
        core_ids=list(range(N_CORES)),
    )
    results = LAST_RESULTS.results

    vals = np.stack([r["out_val"][:, 0] for r in results])  # [8, B]
    idxs = np.stack([r["out_idx"][:, 0] for r in results])  # [8, B]
    win = np.argmax(vals, axis=0)  # first core on ties == smallest global index
    nearest = _unpad_ids(idxs[win, np.arange(B)]) + win * N_LOC
    return Y_train[nearest]


# revision 35
# speedup vs baseline: 3.9524x; 3.9524x over previous
"""Sharded kNN (retrieval) kernel for 8 Trainium2 NeuronCores — v2.

Strategy (classic sharded-kNN reduction, heavily restructured vs v1):
  - Shard X_train / Y_train along N across 8 cores (12500 rows each).
  - Each core computes scores s[b, n] = x_b . t_n - |t_n|^2/2 for its shard
    (argmax of s  <=>  argmin of euclidean distance) via fp16 matmuls on the
    tensor engine.
  - v2 loop order: candidate-block outer (7 blocks of 2000, padded to 2048),
    query-tile inner — X_train streams from HBM exactly ONCE (v1 streamed it
    8x).  Scores are never materialized in SBUF: the DVE MAX8 / FIND_INDEX8
    instructions scan the 4 PSUM banks of each (block, query-tile) directly.
  - Per-block top-8 candidates are merged across blocks by packing the
    bank-padded candidate id (14 bits, < 7*2048 = 14336) into the low
    mantissa bits of the fp32 score (perturbation ~0.2 << top-1/top-2 score
    gap ~7), which makes every merged value unique and lets the final top-8
    positions be recovered with a single bitwise AND - no per-partition
    gather needed.
  - The final 8 candidates per query are gathered from an id-padded DRAM
    table with one batched indirect DMA and re-ranked exactly in fp32
    (identical tensor_tensor dot products + tie-break as v1, so the final
    ordering matches the reference bit-for-bit on ties).
  - Each core outputs (exact best score, padded local argmin) per query; the
    host does the tiny 8-way (min, argmin) reduction and gathers Y_train.
"""

import numpy as np
from contextlib import ExitStack

import concourse.bass as bass
import concourse.mybir as mybir
import concourse.tile as tile
from concourse.bass_utils import run_bass_kernel_spmd

# Problem shape (hardcoded per contest contract).
N_CORES = 8
B = 1024          # queries
D = 768           # feature dim (48*16)
N = 100000        # training rows
N_LOC = N // N_CORES          # 12500 rows per core
P = 128                       # partitions
BT = B // P                   # 8 query tiles
KC = 128                      # contraction tile
KCHUNKS = D // KC             # 6
NBANK = 500                   # real candidates per PSUM bank
BANKPAD = 512                 # PSUM bank stride in fp32 elements (2 KiB)
BLK = 4 * NBANK               # candidates per full block (4 banks)
NBLK = 7                      # 6 full blocks of 2000 + 1 tail block of 500
BLKPAD = 4 * BANKPAD          # padded ids per block (2048)
NPAD = NBLK * BLKPAD          # padded id space (14336 < 2^14)
TOPK = 8
PAD = 776                     # 768 + 1 (t2/2) + 7 zero pad -> 3104B rows
NEG = -60000.0                # pad-score sentinel (fp16-representable)
# Tie-break sentinel: must stay exactly representable in fp32 when combined
# with padded ids < NPAD (so idx - BIG is exact), i.e. well under 2^24.
BIG = 1.0e6
IDMASK = 0x3FFF               # low 14 bits: padded candidate id
VALMASK = 0xFFFFC000          # high bits: fp32 sign/exponent/upper mantissa

_F16 = mybir.dt.float16
_F32 = mybir.dt.float32
_U32 = mybir.dt.uint32

# candidates per bank b (tail block has a single 500-wide bank)
_BANKS = [4] * 6 + [1]


def _split_sync_waits(nc, maxw=1):
    """Workaround for this walrus build: it accepts at most ONE sync-wait
    command per instruction.  Move extra sem waits onto preceding same-engine
    nops (same queue => executed in order before the instruction)."""
    from bass_rust import InstNoOp

    n_split = 0
    for f in nc.m.functions:
        for blk in f.blocks:
            insts = blk.instructions
            i = 0
            while i < len(insts):
                inst = insts[i]
                si = inst.sync_info
                ow = list(si.on_wait) if (si is not None and si.on_wait) else []
                if len(ow) > maxw:
                    keep, extra = ow[-maxw:], ow[:-maxw]
                    inst.sync_info = mybir.SyncInfo(
                        on_wait=keep, on_update=list(si.on_update or [])
                    )
                    nops = []
                    for j in range(0, len(extra), maxw):
                        nop = InstNoOp(name=f"{inst.name}-ws{j}", ins=[], outs=[])
                        nop.engine = inst.engine
                        nop.sync_info = mybir.SyncInfo(
                            on_wait=extra[j : j + maxw], on_update=[]
                        )
                        nops.append(nop)
                    insts[i:i] = nops
                    i += len(nops)
                    n_split += 1
                i += 1
    return n_split


def _build(iters=1, ablate=""):
    """iters>1 repeats the whole pipeline (identical work) inside one NEFF —
    used by the harness to measure true on-HW time differentially.
    ablate: "" full kernel; "noscan" replaces the DVE max8/find_index8 with
    memsets; "mm" additionally drops merge/gather/re-rank (outputs dummy);
    both are for bottleneck attribution only (results are wrong)."""
    nc = bass.Bass()
    xq16 = nc.dram_tensor("xq16", [P, KCHUNKS * B], _F16, kind="ExternalInput")
    xe32 = nc.dram_tensor("xe32", [P, BT * PAD], _F32, kind="ExternalInput")
    xtr16 = nc.dram_tensor("xtr16", [NBLK, P, KCHUNKS * BLKPAD], _F16,
                           kind="ExternalInput")
    xg32 = nc.dram_tensor("xg32", [NPAD, PAD], _F32, kind="ExternalInput")
    out_val = nc.dram_tensor("out_val", [B, 1], _F32, kind="ExternalOutput")
    out_idx = nc.dram_tensor("out_idx", [B, 1], _F32, kind="ExternalOutput")

    with ExitStack() as ctx:
        tc = ctx.enter_context(tile.TileContext(nc))
        const_pool = ctx.enter_context(tc.tile_pool(name="const", bufs=1))
        xtr_pool = ctx.enter_context(tc.tile_pool(name="xtr", bufs=2))
        s16_pool = ctx.enter_context(tc.tile_pool(name="s16", bufs=3))
        top_pool = ctx.enter_context(tc.tile_pool(name="top", bufs=2))
        fin_pool = ctx.enter_context(tc.tile_pool(name="fin", bufs=2))
        gather_pool = ctx.enter_context(tc.tile_pool(name="gather", bufs=2))
        psum_pool = ctx.enter_context(tc.tile_pool(name="psum", bufs=2, space="PSUM"))

        # queries, stationary: [p, k, b] = x_flat[b, k*128+p]
        xq = const_pool.tile([P, KCHUNKS, B], _F16)
        nc.sync.dma_start(xq[:], xq16[:, :])
        # exact fp32 queries (plus -1 marker) for the re-rank
        xe = const_pool.tile([P, BT, PAD], _F32)
        nc.sync.dma_start(xe[:], xe32[:, :])
        # packed (score | id) candidates: [p, bt, blk, 8]
        cand = const_pool.tile([P, BT, NBLK, TOPK], _F32)

        for _rep in range(iters):
            _body(nc, tc, locals(), ablate)

    _split_waits_maybe(nc)
    return nc


def _body(nc, tc, env, ablate=""):
    xq = env["xq"]; xe = env["xe"]
    cand = env["cand"]; xtr16 = env["xtr16"]; xg32 = env["xg32"]
    out_val = env["out_val"]; out_idx = env["out_idx"]
    xtr_pool = env["xtr_pool"]; top_pool = env["top_pool"]
    fin_pool = env["fin_pool"]; gather_pool = env["gather_pool"]
    psum_pool = env["psum_pool"]; s16_pool = env["s16_pool"]
    if True:
        for b in range(NBLK):
            nbank = _BANKS[b]
            xtr = xtr_pool.tile([P, KCHUNKS, BLKPAD], _F16)
            nc.sync.dma_start(xtr[:], xtr16[b, :, :])
            for bt in range(BT):
                bs = slice(bt * P, (bt + 1) * P)
                ps = psum_pool.tile([P, 4, BANKPAD], _F32)
                # k outer: 4 consecutive matmuls share the same stationary
                # weights (one LDWEIGHTS per k-chunk instead of per matmul).
                # The t^2/2 bias rides in xtr's k=5 row 127 (query side 1.0),
                # which also writes the NEG sentinel into the 12 pad columns.
                for k in range(KCHUNKS):
                    for c in range(nbank):
                        nc.tensor.matmul(
                            ps[:, c, :],
                            lhsT=xq[:, k, bs],
                            rhs=xtr[:, k, c * BANKPAD : (c + 1) * BANKPAD],
                            start=(k == 0),
                            stop=(k == KCHUNKS - 1),
                        )
                # ACT drains PSUM to fp16 SBUF (short PSUM exposure), DVE
                # scans SBUF only — avoids PE-vs-DVE PSUM port contention
                s16 = s16_pool.tile([P, 4, BANKPAD], _F16)
                nc.scalar.copy(s16[:, 0:nbank, :], ps[:, 0:nbank, :])
                scan = s16[:, 0:nbank, :].rearrange("p a b -> p (a b)")
                tv = top_pool.tile([P, TOPK], _F16)
                ti = top_pool.tile([P, TOPK], _U32)
                if ablate:
                    nc.vector.memset(tv[:], 1.0)
                    nc.vector.memset(ti[:], 3)
                else:
                    nc.vector.max(tv[:], scan)
                    nc.vector.max_index(ti[:], tv[:], scan)
                # fp16 -> fp32 (exact); low 13 mantissa bits land as zero
                tvf = top_pool.tile([P, TOPK], _F32)
                nc.vector.tensor_copy(tvf[:], tv[:])
                # clear the low 14 mantissa bits of the fp32 score (>>14<<14)
                vm = top_pool.tile([P, TOPK], _U32)
                nc.vector.tensor_scalar(
                    vm[:], tvf[:].bitcast(_U32), 14, 14,
                    op0=mybir.AluOpType.logical_shift_right,
                    op1=mybir.AluOpType.logical_shift_left,
                )
                # global padded id = in-block id | (b * 2048)
                tg = top_pool.tile([P, TOPK], _U32)
                nc.vector.tensor_scalar(
                    tg[:], ti[:], b * BLKPAD, None,
                    op0=mybir.AluOpType.bitwise_or,
                )
                # pack the id into the cleared mantissa bits
                nc.vector.tensor_tensor(
                    cand[:, bt, b, :].bitcast(_U32), vm[:], tg[:],
                    op=mybir.AluOpType.bitwise_or,
                )

        for bt in range(BT):
            bs = slice(bt * P, (bt + 1) * P)
            if ablate == "mm":
                bv = fin_pool.tile([P, 1], _F32)
                nc.vector.memset(bv[:], 0.0)
                bi = fin_pool.tile([P, 1], _F32)
                nc.vector.memset(bi[:], 0.0)
                nc.sync.dma_start(out_val[bs, :], bv[:])
                nc.sync.dma_start(out_idx[bs, :], bi[:])
                continue
            # merge: top-8 of the 56 packed (score|id) values; ids make them
            # unique so FIND_INDEX8 duplicate semantics never matter here
            tp = fin_pool.tile([P, TOPK], _F32)
            nc.vector.max(tp[:], cand[:, bt, :, :].rearrange("p a b -> p (a b)"))
            idx8 = fin_pool.tile([P, TOPK], _U32)
            nc.vector.tensor_scalar(
                idx8[:], tp[:].bitcast(_U32), 18, 18,
                op0=mybir.AluOpType.logical_shift_left,
                op1=mybir.AluOpType.logical_shift_right,
            )

            # gather the 8 candidate rows (768 feats + t2/2 + pad) per query
            # (one indirect DMA per slot: HW SWDGE mis-gathers [128,8] offsets)
            xg = gather_pool.tile([P, TOPK, PAD], _F32)
            for j in range(TOPK):
                nc.gpsimd.indirect_dma_start(
                    out=xg[:, j, :],
                    out_offset=None,
                    in_=xg32[:, :],
                    in_offset=bass.IndirectOffsetOnAxis(ap=idx8[:, j : j + 1], axis=0),
                )

            # exact fp32 re-rank: cand8[j] = xe . xg[j] = x.t - t2/2
            cand8 = fin_pool.tile([P, TOPK], _F32)
            scratch = gather_pool.tile([P, PAD], _F32)
            for j in range(TOPK):
                nc.vector.scalar_tensor_tensor(
                    out=scratch[:],
                    in0=xg[:, j, :],
                    scalar=0.0,
                    in1=xe[:, bt, :],
                    op0=mybir.AluOpType.add,
                    op1=mybir.AluOpType.mult,
                    accum_out=cand8[:, j : j + 1],
                )

            bv = fin_pool.tile([P, 1], _F32)
            nc.vector.tensor_reduce(
                bv[:], cand8[:], axis=mybir.AxisListType.X, op=mybir.AluOpType.max
            )
            # pick the smallest padded id among exact-score ties:
            # masked = tif + BIG*(cand8 != bv), then min
            tif = fin_pool.tile([P, TOPK], _F32)
            nc.vector.tensor_copy(tif[:], idx8[:])
            neq = fin_pool.tile([P, TOPK], _F32)
            nc.vector.tensor_scalar(
                neq[:], cand8[:], bv[:], None, op0=mybir.AluOpType.not_equal
            )
            masked = fin_pool.tile([P, TOPK], _F32)
            nc.vector.scalar_tensor_tensor(
                masked[:],
                in0=neq[:],
                scalar=BIG,
                in1=tif[:],
                op0=mybir.AluOpType.mult,
                op1=mybir.AluOpType.add,
            )
            bi = fin_pool.tile([P, 1], _F32)
            nc.vector.tensor_reduce(
                bi[:], masked[:], axis=mybir.AxisListType.X, op=mybir.AluOpType.min
            )

            nc.sync.dma_start(out_val[bs, :], bv[:])
            nc.sync.dma_start(out_idx[bs, :], bi[:])


def _split_waits_maybe(nc):
    import os
    if not os.environ.get("BASS_NO_SPLIT_WAITS"):
        _split_sync_waits(nc)


_NC_CACHE = None
LAST_RESULTS = None  # BassKernelResults of the most recent run (for test harness)

# map padded id -> local row: n = 2000*(g//2048) + 500*((g%2048)//512) + g%512
def _unpad_ids(g):
    g = np.asarray(g, dtype=np.int64)
    blk, rem = np.divmod(g, BLKPAD)
    c, i = np.divmod(rem, BANKPAD)
    return blk * BLK + c * NBANK + i


def prepare_in_maps(x, X_train):
    x = np.asarray(x, dtype=np.float32)
    X_train = np.asarray(X_train, dtype=np.float32)

    x_flat = np.ascontiguousarray(x.reshape(B, D))
    xt16 = x_flat.astype(np.float16)  # [B, D]
    # [p, k, b] = x16[b, k*128+p]; slot (k=5, p=127) carries the bias
    # constant 1.0 instead of feature 767 (dropped from the selection score;
    # the exact re-rank still uses all 768 features)
    xq16 = np.ascontiguousarray(
        xt16.reshape(B, KCHUNKS, P).transpose(2, 1, 0)
    )
    xq16[P - 1, KCHUNKS - 1, :] = np.float16(1.0)
    xq16 = xq16.reshape(P, KCHUNKS * B)
    # [p, bt, d] = xe[bt*128+p, d]
    xe = np.concatenate(
        [x_flat, -np.ones((B, 1), np.float32), np.zeros((B, PAD - D - 1), np.float32)],
        axis=1,
    )
    xe32 = np.ascontiguousarray(
        xe.reshape(BT, P, PAD).transpose(1, 0, 2)
    ).reshape(P, BT * PAD)

    # local row n -> padded id
    n_loc = np.arange(N_LOC)
    blk, rem = np.divmod(n_loc, BLK)
    c, i = np.divmod(rem, NBANK)
    gids = blk * BLKPAD + c * BANKPAD + i  # [N_LOC]

    in_maps = []
    for core in range(N_CORES):
        Xc = X_train[core * N_LOC : (core + 1) * N_LOC]
        t2 = (Xc.astype(np.float64) ** 2).sum(axis=1)
        X16 = Xc.astype(np.float16)  # [N_LOC, D]

        # xtr16[b, p, k*2048 + g] = X16[n(b,g), k*128+p], pad slots zero.
        # Slot (k=5, p=127) carries the bias row (t2.mean - t2)/2 in place of
        # feature 767, with the NEG sentinel in the pad columns.
        xtr = np.zeros((NBLK, P, KCHUNKS, BLKPAD), np.float16)
        x16v = X16.reshape(N_LOC, KCHUNKS, P)  # [n, k, p]
        xtr[blk, :, :, rem // NBANK * BANKPAD + rem % NBANK] = x16v.transpose(0, 2, 1)
        vrow = np.full((NBLK, BLKPAD), NEG, np.float16).reshape(-1)
        vrow[gids] = ((t2.mean() - t2) * 0.5).astype(np.float16)
        xtr[:, P - 1, KCHUNKS - 1, :] = vrow.reshape(NBLK, BLKPAD)
        xtr16 = np.ascontiguousarray(xtr).reshape(NBLK, P, KCHUNKS * BLKPAD)

        xg32 = np.zeros((NPAD, PAD), np.float32)
        xg32[gids, :D] = Xc
        xg32[gids, D] = (t2 * 0.5).astype(np.float32)

        in_maps.append(
            {
                "xq16": xq16,
                "xe32": xe32,
                "xtr16": xtr16,
                "xg32": np.ascontiguousarray(xg32),
            }
        )
    return in_maps


def kernel(x, X_train, Y_train):
    global _NC_CACHE, LAST_RESULTS
    Y_train = np.asarray(Y_train)
    in_maps = prepare_in_maps(x, X_train)

    if _NC_CACHE is None:
        _NC_CACHE = _build()

    LAST_RESULTS = run_bass_kernel_spmd(
        _NC_CACHE,
        in_maps,
        core_ids=list(range(N_CORES)),
    )
    results = LAST_RESULTS.results

    vals = np.stack([r["out_val"][:, 0] for r in results])  # [8, B]
    idxs = np.stack([r["out_idx"][:, 0] for r in results])  # [8, B]
    win = np.argmax(vals, axis=0)  # first core on ties == smallest global index
    nearest = _unpad_ids(idxs[win, np.arange(B)]) + win * N_LOC
    return Y_train[nearest]


# revision 40
# speedup vs baseline: 8.2108x; 2.0774x over previous
"""Sharded kNN (retrieval) kernel for 8 Trainium2 NeuronCores — v2.

Strategy (classic sharded-kNN reduction, heavily restructured vs v1):
  - Shard X_train / Y_train along N across 8 cores (12500 rows each).
  - Each core computes scores s[b, n] = x_b . t_n - |t_n|^2/2 for its shard
    (argmax of s  <=>  argmin of euclidean distance) via fp16 matmuls on the
    tensor engine.
  - v2 loop order: candidate-block outer (7 blocks of 2000, padded to 2048),
    query-tile inner — X_train streams from HBM exactly ONCE (v1 streamed it
    8x).  Scores are never materialized in SBUF: the DVE MAX8 / FIND_INDEX8
    instructions scan the 4 PSUM banks of each (block, query-tile) directly.
  - Per-block top-8 candidates are merged across blocks by packing the
    bank-padded candidate id (14 bits, < 7*2048 = 14336) into the low
    mantissa bits of the fp32 score (perturbation ~0.2 << top-1/top-2 score
    gap ~7), which makes every merged value unique and lets the final top-8
    positions be recovered with a single bitwise AND - no per-partition
    gather needed.
  - The final 8 candidates per query are gathered from an id-padded DRAM
    table with one batched indirect DMA and re-ranked exactly in fp32
    (identical tensor_tensor dot products + tie-break as v1, so the final
    ordering matches the reference bit-for-bit on ties).
  - Each core outputs (exact best score, padded local argmin) per query; the
    host does the tiny 8-way (min, argmin) reduction and gathers Y_train.
"""

import numpy as np
from contextlib import ExitStack

import concourse.bass as bass
import concourse.mybir as mybir
import concourse.tile as tile
from concourse.bass_utils import run_bass_kernel_spmd

# Problem shape (hardcoded per contest contract).
N_CORES = 8
B = 1024          # queries
D = 768           # feature dim (48*16)
N = 100000        # training rows
N_LOC = N // N_CORES          # 12500 rows per core
P = 128                       # partitions
BT = B // P                   # 8 query tiles
KC = 128                      # contraction tile
KCHUNKS = D // KC             # 6
NBANK = 500                   # real candidates per PSUM bank
BANKPAD = 512                 # PSUM bank stride in fp32 elements (2 KiB)
BLK = 4 * NBANK               # candidates per full block (4 banks)
NBLK = 7                      # 6 full blocks of 2000 + 1 tail block of 500
BLKPAD = 4 * BANKPAD          # padded ids per block (2048)
NPAD = NBLK * BLKPAD          # padded id space (14336 < 2^14)
TOPK = 8
PAD = 776                     # 768 + 1 (t2/2) + 7 zero pad -> 3104B rows
NEG = -60000.0                # pad-score sentinel (fp16-representable)
# Tie-break sentinel: must stay exactly representable in fp32 when combined
# with padded ids < NPAD (so idx - BIG is exact), i.e. well under 2^24.
BIG = 1.0e6
IDMASK = 0x3FFF               # low 14 bits: padded candidate id
VALMASK = 0xFFFFC000          # high bits: fp32 sign/exponent/upper mantissa

_F16 = mybir.dt.float16
_F32 = mybir.dt.float32
_U32 = mybir.dt.uint32

# candidates per bank b (tail block has a single 500-wide bank)
_BANKS = [4] * 6 + [1]
# DVE scan groups: pairs of full blocks (4096 wide) + the 512-wide tail
NGRP = 4


def _split_sync_waits(nc, maxw=1):
    """Workaround for this walrus build: it accepts at most ONE sync-wait
    command per instruction.  Move extra sem waits onto preceding same-engine
    nops (same queue => executed in order before the instruction)."""
    from bass_rust import InstNoOp

    n_split = 0
    for f in nc.m.functions:
        for blk in f.blocks:
            insts = blk.instructions
            i = 0
            while i < len(insts):
                inst = insts[i]
                si = inst.sync_info
                ow = list(si.on_wait) if (si is not None and si.on_wait) else []
                if len(ow) > maxw:
                    keep, extra = ow[-maxw:], ow[:-maxw]
                    inst.sync_info = mybir.SyncInfo(
                        on_wait=keep, on_update=list(si.on_update or [])
                    )
                    nops = []
                    for j in range(0, len(extra), maxw):
                        nop = InstNoOp(name=f"{inst.name}-ws{j}", ins=[], outs=[])
                        nop.engine = inst.engine
                        nop.sync_info = mybir.SyncInfo(
                            on_wait=extra[j : j + maxw], on_update=[]
                        )
                        nops.append(nop)
                    insts[i:i] = nops
                    i += len(nops)
                    n_split += 1
                i += 1
    return n_split


def _build(iters=1, ablate=""):
    """iters>1 repeats the whole pipeline (identical work) inside one NEFF —
    used by the harness to measure true on-HW time differentially.
    ablate: "" full kernel; "noscan" replaces the DVE max8/find_index8 with
    memsets; "mm" additionally drops merge/gather/re-rank (outputs dummy);
    both are for bottleneck attribution only (results are wrong)."""
    nc = bass.Bass()
    xq16 = nc.dram_tensor("xq16", [P, KCHUNKS * B], _F16, kind="ExternalInput")
    xe32 = nc.dram_tensor("xe32", [P, BT * PAD], _F32, kind="ExternalInput")
    xtr16 = nc.dram_tensor("xtr16", [NBLK, P, KCHUNKS * BLKPAD], _F16,
                           kind="ExternalInput")
    xg32 = nc.dram_tensor("xg32", [NPAD, PAD], _F32, kind="ExternalInput")
    out_val = nc.dram_tensor("out_val", [B, 1], _F32, kind="ExternalOutput")
    out_idx = nc.dram_tensor("out_idx", [B, 1], _F32, kind="ExternalOutput")

    with ExitStack() as ctx:
        tc = ctx.enter_context(tile.TileContext(nc))
        const_pool = ctx.enter_context(tc.tile_pool(name="const", bufs=1))
        xtr_pool = ctx.enter_context(tc.tile_pool(name="xtr", bufs=2))
        s16_pool = ctx.enter_context(tc.tile_pool(name="s16", bufs=8))
        top_pool = ctx.enter_context(tc.tile_pool(name="top", bufs=2))
        fin_pool = ctx.enter_context(tc.tile_pool(name="fin", bufs=2))
        gather_pool = ctx.enter_context(tc.tile_pool(name="gather", bufs=1))
        psum_pool = ctx.enter_context(tc.tile_pool(name="psum", bufs=2, space="PSUM"))

        # queries, stationary: [p, k, b] = x_flat[b, k*128+p]
        xq = const_pool.tile([P, KCHUNKS, B], _F16)
        nc.sync.dma_start(xq[:], xq16[:, :])
        # exact fp32 queries (plus -1 marker) for the re-rank
        xe = const_pool.tile([P, BT, PAD], _F32)
        nc.sync.dma_start(xe[:], xe32[:, :])
        # packed (score | id) candidates: [p, bt, scan-group, 8]
        cand = const_pool.tile([P, BT, NGRP, TOPK], _F32)

        for _rep in range(iters):
            _body(nc, tc, locals(), ablate)

    _split_waits_maybe(nc)
    return nc


def _body(nc, tc, env, ablate=""):
    xq = env["xq"]; xe = env["xe"]
    cand = env["cand"]; xtr16 = env["xtr16"]; xg32 = env["xg32"]
    out_val = env["out_val"]; out_idx = env["out_idx"]
    xtr_pool = env["xtr_pool"]; top_pool = env["top_pool"]
    fin_pool = env["fin_pool"]; gather_pool = env["gather_pool"]
    psum_pool = env["psum_pool"]; s16_pool = env["s16_pool"]
    s16_by_bt = {}
    if True:
        for b in range(NBLK):
            nbank = _BANKS[b]
            xtr = xtr_pool.tile([P, KCHUNKS, BLKPAD], _F16)
            nc.sync.dma_start(xtr[:], xtr16[b, :, :])
            for bt in range(BT):
                bs = slice(bt * P, (bt + 1) * P)
                ps = psum_pool.tile([P, 4, BANKPAD], _F32)
                # k outer: 4 consecutive matmuls share the same stationary
                # weights (one LDWEIGHTS per k-chunk instead of per matmul).
                # The t^2/2 bias rides in xtr's k=5 row 127 (query side 1.0),
                # which also writes the NEG sentinel into the 12 pad columns.
                for k in range(KCHUNKS):
                    for c in range(nbank):
                        nc.tensor.matmul(
                            ps[:, c, :],
                            lhsT=xq[:, k, bs],
                            rhs=xtr[:, k, c * BANKPAD : (c + 1) * BANKPAD],
                            start=(k == 0),
                            stop=(k == KCHUNKS - 1),
                        )
                # ACT drains PSUM to fp16 SBUF (short PSUM exposure); two
                # consecutive blocks share one s16 tile so the DVE scans
                # 4096-wide groups — half the scan instructions and pack ops
                grp, half = divmod(b, 2)
                if half == 0:
                    s16 = s16_pool.tile([P, 2, 4, BANKPAD], _F16)
                    s16_by_bt[bt] = s16
                else:
                    s16 = s16_by_bt[bt]
                nc.scalar.copy(s16[:, half, 0:nbank, :], ps[:, 0:nbank, :])
                if b < NBLK - 1 and half == 0:
                    continue  # scan fires once per pair (or for the tail)
                width = 2 * 4 * BANKPAD if half == 1 else BANKPAD
                scan = s16.rearrange("p a b c -> p (a b c)")[:, 0:width]
                tv = top_pool.tile([P, TOPK], _F16)
                ti = top_pool.tile([P, TOPK], _U32)
                if ablate:
                    nc.vector.memset(tv[:], 1.0)
                    nc.vector.memset(ti[:], 3)
                else:
                    nc.vector.max(tv[:], scan)
                    nc.vector.max_index(ti[:], tv[:], scan)
                # fp16 -> fp32 (exact); low 13 mantissa bits land as zero
                tvf = top_pool.tile([P, TOPK], _F32)
                nc.vector.tensor_copy(tvf[:], tv[:])
                # clear the low 14 mantissa bits of the fp32 score (>>14<<14)
                vm = top_pool.tile([P, TOPK], _U32)
                nc.vector.tensor_scalar(
                    vm[:], tvf[:].bitcast(_U32), 14, 14,
                    op0=mybir.AluOpType.logical_shift_right,
                    op1=mybir.AluOpType.logical_shift_left,
                )
                # global padded id = in-group id | (grp * 4096)
                tg = top_pool.tile([P, TOPK], _U32)
                nc.vector.tensor_scalar(
                    tg[:], ti[:], grp * 2 * BLKPAD, None,
                    op0=mybir.AluOpType.bitwise_or,
                )
                # pack the id into the cleared mantissa bits
                nc.vector.tensor_tensor(
                    cand[:, bt, grp, :].bitcast(_U32), vm[:], tg[:],
                    op=mybir.AluOpType.bitwise_or,
                )

        for bt in range(BT):
            bs = slice(bt * P, (bt + 1) * P)
            if ablate == "mm":
                bv = fin_pool.tile([P, 1], _F32)
                nc.vector.memset(bv[:], 0.0)
                bi = fin_pool.tile([P, 1], _F32)
                nc.vector.memset(bi[:], 0.0)
                nc.sync.dma_start(out_val[bs, :], bv[:])
                nc.sync.dma_start(out_idx[bs, :], bi[:])
                continue
            # merge: top-8 of the 56 packed (score|id) values; ids make them
            # unique so FIND_INDEX8 duplicate semantics never matter here
            tp = fin_pool.tile([P, TOPK], _F32)
            nc.vector.max(tp[:], cand[:, bt, :, :].rearrange("p a b -> p (a b)"))
            idx8 = fin_pool.tile([P, TOPK], _U32)
            nc.vector.tensor_scalar(
                idx8[:], tp[:].bitcast(_U32), 18, 18,
                op0=mybir.AluOpType.logical_shift_left,
                op1=mybir.AluOpType.logical_shift_right,
            )

            # gather the 8 candidate rows (768 feats + t2/2 + pad) per query
            # (one indirect DMA per slot: HW SWDGE mis-gathers [128,8] offsets)
            xg = gather_pool.tile([P, TOPK, PAD], _F32)
            for j in range(TOPK):
                nc.gpsimd.indirect_dma_start(
                    out=xg[:, j, :],
                    out_offset=None,
                    in_=xg32[:, :],
                    in_offset=bass.IndirectOffsetOnAxis(ap=idx8[:, j : j + 1], axis=0),
                )

            # exact fp32 re-rank: cand8[j] = xe . xg[j] = x.t - t2/2
            cand8 = fin_pool.tile([P, TOPK], _F32)
            scratch = gather_pool.tile([P, PAD], _F32)
            for j in range(TOPK):
                nc.vector.scalar_tensor_tensor(
                    out=scratch[:],
                    in0=xg[:, j, :],
                    scalar=0.0,
                    in1=xe[:, bt, :],
                    op0=mybir.AluOpType.add,
                    op1=mybir.AluOpType.mult,
                    accum_out=cand8[:, j : j + 1],
                )

            bv = fin_pool.tile([P, 1], _F32)
            nc.vector.tensor_reduce(
                bv[:], cand8[:], axis=mybir.AxisListType.X, op=mybir.AluOpType.max
            )
            # pick the smallest padded id among exact-score ties:
            # masked = tif + BIG*(cand8 != bv), then min
            tif = fin_pool.tile([P, TOPK], _F32)
            nc.vector.tensor_copy(tif[:], idx8[:])
            neq = fin_pool.tile([P, TOPK], _F32)
            nc.vector.tensor_scalar(
                neq[:], cand8[:], bv[:], None, op0=mybir.AluOpType.not_equal
            )
            masked = fin_pool.tile([P, TOPK], _F32)
            nc.vector.scalar_tensor_tensor(
                masked[:],
                in0=neq[:],
                scalar=BIG,
                in1=tif[:],
                op0=mybir.AluOpType.mult,
                op1=mybir.AluOpType.add,
            )
            bi = fin_pool.tile([P, 1], _F32)
            nc.vector.tensor_reduce(
                bi[:], masked[:], axis=mybir.AxisListType.X, op=mybir.AluOpType.min
            )

            nc.sync.dma_start(out_val[bs, :], bv[:])
            nc.sync.dma_start(out_idx[bs, :], bi[:])


def _split_waits_maybe(nc):
    import os
    if not os.environ.get("BASS_NO_SPLIT_WAITS"):
        _split_sync_waits(nc)


_NC_CACHE = None
LAST_RESULTS = None  # BassKernelResults of the most recent run (for test harness)

# map padded id -> local row: n = 2000*(g//2048) + 500*((g%2048)//512) + g%512
def _unpad_ids(g):
    g = np.asarray(g, dtype=np.int64)
    blk, rem = np.divmod(g, BLKPAD)
    c, i = np.divmod(rem, BANKPAD)
    return blk * BLK + c * NBANK + i


def prepare_in_maps(x, X_train):
    x = np.asarray(x, dtype=np.float32)
    X_train = np.asarray(X_train, dtype=np.float32)

    x_flat = np.ascontiguousarray(x.reshape(B, D))
    xt16 = x_flat.astype(np.float16)  # [B, D]
    # [p, k, b] = x16[b, k*128+p]; slot (k=5, p=127) carries the bias
    # constant 1.0 instead of feature 767 (dropped from the selection score;
    # the exact re-rank still uses all 768 features)
    xq16 = np.ascontiguousarray(
        xt16.reshape(B, KCHUNKS, P).transpose(2, 1, 0)
    )
    xq16[P - 1, KCHUNKS - 1, :] = np.float16(1.0)
    xq16 = xq16.reshape(P, KCHUNKS * B)
    # [p, bt, d] = xe[bt*128+p, d]
    xe = np.concatenate(
        [x_flat, -np.ones((B, 1), np.float32), np.zeros((B, PAD - D - 1), np.float32)],
        axis=1,
    )
    xe32 = np.ascontiguousarray(
        xe.reshape(BT, P, PAD).transpose(1, 0, 2)
    ).reshape(P, BT * PAD)

    # local row n -> padded id
    n_loc = np.arange(N_LOC)
    blk, rem = np.divmod(n_loc, BLK)
    c, i = np.divmod(rem, NBANK)
    gids = blk * BLKPAD + c * BANKPAD + i  # [N_LOC]

    in_maps = []
    for core in range(N_CORES):
        Xc = X_train[core * N_LOC : (core + 1) * N_LOC]
        t2 = (Xc.astype(np.float64) ** 2).sum(axis=1)
        X16 = Xc.astype(np.float16)  # [N_LOC, D]

        # xtr16[b, p, k*2048 + g] = X16[n(b,g), k*128+p], pad slots zero.
        # Slot (k=5, p=127) carries the bias row (t2.mean - t2)/2 in place of
        # feature 767, with the NEG sentinel in the pad columns.
        xtr = np.zeros((NBLK, P, KCHUNKS, BLKPAD), np.float16)
        x16v = X16.reshape(N_LOC, KCHUNKS, P)  # [n, k, p]
        xtr[blk, :, :, rem // NBANK * BANKPAD + rem % NBANK] = x16v.transpose(0, 2, 1)
        vrow = np.full((NBLK, BLKPAD), NEG, np.float16).reshape(-1)
        vrow[gids] = ((t2.mean() - t2) * 0.5).astype(np.float16)
        xtr[:, P - 1, KCHUNKS - 1, :] = vrow.reshape(NBLK, BLKPAD)
        xtr16 = np.ascontiguousarray(xtr).reshape(NBLK, P, KCHUNKS * BLKPAD)

        xg32 = np.zeros((NPAD, PAD), np.float32)
        xg32[gids, :D] = Xc
        xg32[gids, D] = (t2 * 0.5).astype(np.float32)

        in_maps.append(
            {
                "xq16": xq16,
                "xe32": xe32,
                "xtr16": xtr16,
                "xg32": np.ascontiguousarray(xg32),
            }
        )
    return in_maps


def kernel(x, X_train, Y_train):
    global _NC_CACHE, LAST_RESULTS
    Y_train = np.asarray(Y_train)
    in_maps = prepare_in_maps(x, X_train)

    if _NC_CACHE is None:
        _NC_CACHE = _build()

    LAST_RESULTS = run_bass_kernel_spmd(
        _NC_CACHE,
        in_maps,
        core_ids=list(range(N_CORES)),
    )
    results = LAST_RESULTS.results

    vals = np.stack([r["out_val"][:, 0] for r in results])  # [8, B]
    idxs = np.stack([r["out_idx"][:, 0] for r in results])  # [8, B]
    win = np.argmax(vals, axis=0)  # first core on ties == smallest global index
    nearest = _unpad_ids(idxs[win, np.arange(B)]) + win * N_LOC
    return Y_train[nearest]
